# revision 8
# baseline (speedup 1.0000x reference)
"""Capsule routing pooling kernel for Trainium2 (8 NeuronCores, data parallel).

Math: the reference's softmax is over a singleton axis, so the routing
coefficients are identically 1.0 and the routing iterations never affect the
output.  The computation reduces to, per (b, c, 2x2 spatial tile):
    s   = sum of the four D=16 vectors in the tile
    sq  = sum_d s_d^2
    out = s * sq / ((1 + sq) * (sqrt(sq) + 1e-8)) = s * sqrt(sq) / (1 + sq)

Sharding: batch dim (16) split across 8 cores -> 2 batches/core.  Per core the
(2*64)=128 (b,c) pairs map onto the 128 SBUF partitions; each partition owns a
full 64x64x16 image.

Per-core pipeline (super-groups of row-pairs, schedule [8,8,8,4,2,1,1]):
  - 4 input rows per 2 MB load (contiguous per partition, HWDGE on nc.sync)
  - row-pair add (fp32 in -> fp16 out, DVE 1x) then column-pair add (fp16,
    DVE 2x packed mode) -- fp16 intermediates halve most DVE work vs fp32
    and are ~10x more precise than bf16 for this value range
  - sum over D via a 4-level pairwise tree add (3 fp16 2x levels + 1 fp32
    level) instead of tensor_reduce, which is hard-capped at 1x on DVE
  - square on ACT (fp16 SBUF out; PSUM would cap downstream DVE at 1x),
    emitted immediately after each super-group's col add so ACT always runs
    one super-group ahead of the DVE tail work that consumes it
  - scale chain: sqrt + (1+sq) on ACT, reciprocal + mul on DVE
  - final broadcast multiply on DVE (1x; broadcast operand blocks packing),
    fp16 output stored via the nc.scalar HWDGE ring; host upcasts to fp32
    (exact re-encoding, values unchanged) -- halves store HBM traffic
  - each super-group's tail is emitted one super-group late so the in-order
    DVE stream never waits on ACT's square at the boundary; small tail
    super-groups shorten the post-last-load drain.
"""

import numpy as np

import concourse.bass as bass
import concourse.bacc as bacc
import concourse.tile as tile
from concourse import mybir
from concourse.bass_utils import run_bass_kernel_spmd

_B, _C, _H, _W, _D = 16, 64, 64, 64, 16
_NCORES = 8
_F32 = mybir.dt.float32
_F16 = mybir.dt.float16
_OUT_F16 = True  # store fp16 on device, exact upcast to fp32 on host


def _kernel_body(tc, out_ap, in_ap, H, W, D):
    nc = tc.nc
    P = 128
    nH, nW = H // 2, W // 2
    out_dt = _F16 if _OUT_F16 else _F32

    inv4 = in_ap.rearrange("p (q four) w d -> p q (four w d)", four=4)
    inv2 = in_ap.rearrange("p (rp two) w d -> p rp (two w d)", two=2)
    outv = out_ap.rearrange("p y x d -> p y (x d)")

    # super-group schedule in row-pair units: small batches at both ends
    # (fast pipeline fill / short drain tail), big in the middle
    if nH >= 32:
        sched = [8] * ((nH - 16) // 8) + [8, 4, 2, 1, 1]
    elif nH >= 8:
        sched = [8] * (nH // 8)
    else:
        sched = [nH]
    assert sum(sched) == nH
    nsg_max = max(sched) * nW

    import contextlib
    import itertools

    # Loads rotate across DMA-capable rings: one ring generates descriptors
    # at only ~20M/s (~320 GB/s of 16KB chunks), which throttles the startup
    # ramp and adds head-of-line blocking when a load waits on a slab buffer.
    # The scalar ring joins only for the first super-group's loads -- it must
    # stay clear of stores later (loads queued behind a compute-blocked store
    # would stall).
    early_rings = itertools.cycle([nc.sync, nc.gpsimd, nc.scalar])
    main_rings = itertools.cycle([nc.sync, nc.gpsimd])

    with contextlib.ExitStack() as ctx:
        slabs = ctx.enter_context(tc.tile_pool(name="slabs", bufs=6))
        rpool = ctx.enter_context(tc.tile_pool(name="rpool", bufs=2))
        mid = ctx.enter_context(tc.tile_pool(name="mid", bufs=3))
        sqp = ctx.enter_context(tc.tile_pool(name="sqp", bufs=2))
        treep = ctx.enter_context(tc.tile_pool(name="treep", bufs=2))
        small = ctx.enter_context(tc.tile_pool(name="small", bufs=2))
        outp = ctx.enter_context(tc.tile_pool(name="outp", bufs=2))

        # one queued tail per super-group, emitted one SG late so the DVE
        # instruction stream never waits on ACT's square at SG boundaries
        pending = []

        def emit_front(sg, g0, fine=False, early=False):
            """loads + row-pair adds + column-pair adds for one super-group
            of `sg` row-pairs starting at output row g0.  fine=True loads one
            row-pair per DMA (1 MB) for fast pipeline fill."""
            load_rings = early_rings if early else main_rings
            s_sg = mid.tile([P, sg, nW, D], _F16, tag="s_sg")
            for ci in range(0, sg, 4):
                cg = min(4, sg - ci)  # row-pairs this col-add batch
                r = rpool.tile([P, 4, nW, 2, D], _F16, tag="r")
                for li in range(0, cg, 2):
                    if fine:
                        for q in range(min(2, cg - li)):
                            rp = g0 + ci + li + q
                            slab = slabs.tile([P, 1, 2, nW, 2, D], _F32, tag="slab")
                            next(load_rings).dma_start(
                                out=slab[:],
                                in_=inv2[:, rp, :].rearrange(
                                    "p (two b) -> p two b", two=2
                                ),
                            )
                            nc.vector.tensor_add(
                                r[:, li + q : li + q + 1, :, :, :],
                                slab[:, :, 0, :, :, :],
                                slab[:, :, 1, :, :, :],
                            )
                        continue
                    t = (g0 + ci + li) // 2
                    slab = slabs.tile([P, 2, 2, nW, 2, D], _F32, tag="slab")
                    next(load_rings).dma_start(
                        out=slab[:],
                        in_=inv4[:, t, :].rearrange(
                            "p (a two b) -> p a two b", a=2, two=2
                        ),
                    )
                    # row-pair sums for 2 row-pairs (DVE 1x: fp32 src)
                    nc.vector.tensor_add(
                        r[:, li : li + 2, :, :, :],
                        slab[:, :, 0, :, :, :],
                        slab[:, :, 1, :, :, :],
                    )
                # column-pair add for cg row-pairs (DVE 2x: all fp16)
                nc.vector.tensor_add(
                    s_sg[:, ci : ci + cg, :, :],
                    r[:, 0:cg, :, 0, :],
                    r[:, 0:cg, :, 1, :],
                )
            return s_sg

        def emit_square(s_sg, sg):
            """ACT square for one super-group, emitted right after its front
            so ACT runs a full super-group ahead of the DVE tail."""
            nsg = sg * nW
            sv = s_sg[:].rearrange("p s x d -> p (s x) d")
            s2 = sqp.tile([P, nsg_max, D], _F16, tag="s2")
            nc.scalar.activation(
                s2[:, 0:nsg, :], sv, mybir.ActivationFunctionType.Square
            )
            return s2

        def tree_reduce(s2, nsg):
            """sum over D=16 via pairwise tree: 3 fp16 2x levels + 1 fp32."""
            t1 = treep.tile([P, nsg_max, 8], _F16, tag="t1")
            t2 = treep.tile([P, nsg_max, 4], _F16, tag="t2")
            t3 = treep.tile([P, nsg_max, 2], _F16, tag="t3")
            nc.vector.tensor_add(
                t1[:, 0:nsg, :], s2[:, 0:nsg, 0:8], s2[:, 0:nsg, 8:16]
            )
            nc.vector.tensor_add(
                t2[:, 0:nsg, :], t1[:, 0:nsg, 0:4], t1[:, 0:nsg, 4:8]
            )
            nc.vector.tensor_add(
                t3[:, 0:nsg, :], t2[:, 0:nsg, 0:2], t2[:, 0:nsg, 2:4]
            )
            return t3

        def chain_slots(nsg):
            ch = small.tile([P, nsg_max, 4], _F32, tag="ch")
            return (
                ch[:, 0:nsg, 0:1],  # sq
                ch[:, 0:nsg, 1:2],  # c1 = 1 + sq
                ch[:, 0:nsg, 2:3],  # a = sqrt(sq)
                ch[:, 0:nsg, 3:4],  # rec = 1/(1+sq); then sc = a*rec
            )

        def emit_tail(sg, g0, s_sg, s2):
            """tree + squash scale + final multiply + store for one SG."""
            nsg = sg * nW
            sv = s_sg[:].rearrange("p s x d -> p (s x) d")
            t3 = tree_reduce(s2, nsg)
            sq, c1, a, rec = chain_slots(nsg)
            nc.vector.tensor_add(sq, t3[:, 0:nsg, 0:1], t3[:, 0:nsg, 1:2])
            nc.scalar.add(c1, sq, 1.0)
            nc.scalar.activation(a, sq, mybir.ActivationFunctionType.Sqrt)
            nc.vector.reciprocal_approx_fast(rec, c1)
            sc = c1  # reuse the c1 slot for the final scale
            nc.vector.tensor_mul(sc, a, rec)
            out_t = outp.tile([P, nsg_max, D], out_dt, tag="out")
            nc.vector.tensor_mul(
                out_t[:, 0:nsg, :], sv, sc.to_broadcast((P, nsg, D))
            )
            nc.scalar.dma_start(
                out=outv[:, g0 : g0 + sg, :],
                in_=out_t[:, 0:nsg, :].rearrange("p n d -> p (n d)"),
            )

        def emit_tail2(tp1, tp2):
            """the last two tails, op-interleaved so ACT and DVE pipeline
            instead of ping-ponging through two serial chains."""
            (sg1, g01, s1, q1), (sg2, g02, s2_, q2) = tp1, tp2
            n1, n2 = sg1 * nW, sg2 * nW
            sv1 = s1[:].rearrange("p s x d -> p (s x) d")
            sv2 = s2_[:].rearrange("p s x d -> p (s x) d")
            u3 = tree_reduce(q1, n1)
            sqa, c1a, aa, reca = chain_slots(n1)
            nc.vector.tensor_add(sqa, u3[:, 0:n1, 0:1], u3[:, 0:n1, 1:2])
            nc.scalar.add(c1a, sqa, 1.0)
            nc.scalar.activation(aa, sqa, mybir.ActivationFunctionType.Sqrt)
            v3 = tree_reduce(q2, n2)
            sqb, c1b, ab, recb = chain_slots(n2)
            nc.vector.tensor_add(sqb, v3[:, 0:n2, 0:1], v3[:, 0:n2, 1:2])
            nc.vector.reciprocal_approx_fast(reca, c1a)
            nc.scalar.add(c1b, sqb, 1.0)
            nc.scalar.activation(ab, sqb, mybir.ActivationFunctionType.Sqrt)
            sca = c1a
            nc.vector.tensor_mul(sca, aa, reca)
            o1 = outp.tile([P, nsg_max, D], out_dt, tag="out")
            nc.vector.tensor_mul(
                o1[:, 0:n1, :], sv1, sca.to_broadcast((P, n1, D))
            )
            nc.scalar.dma_start(
                out=outv[:, g01 : g01 + sg1, :],
                in_=o1[:, 0:n1, :].rearrange("p n d -> p (n d)"),
            )
            nc.vector.reciprocal_approx_fast(recb, c1b)
            scb = c1b
            nc.vector.tensor_mul(scb, ab, recb)
            o2 = outp.tile([P, nsg_max, D], out_dt, tag="out")
            nc.vector.tensor_mul(
                o2[:, 0:n2, :], sv2, scb.to_broadcast((P, n2, D))
            )
            nc.scalar.dma_start(
                out=outv[:, g02 : g02 + sg2, :],
                in_=o2[:, 0:n2, :].rearrange("p n d -> p (n d)"),
            )

        g0 = 0
        last = len(sched) - 1
        for si, sg in enumerate(sched):
            fine = len(sched) > 2 and (si == 0 or si >= len(sched) - 2)
            front = emit_front(sg, g0, fine=fine, early=(si == 0))
            s2 = emit_square(front, sg)
            if pending and si < last:
                emit_tail(*pending.pop(0))
            pending.append((sg, g0, front, s2))
            g0 += sg
        if len(pending) == 2:
            emit_tail2(pending[0], pending[1])
        else:
            for t in pending:
                emit_tail(*t)


def build_nc(H=_H, W=_W, D=_D):
    """Build and compile the per-core Bass program."""
    nc = bacc.Bacc("TRN2", target_bir_lowering=False, debug=False)
    inp = nc.dram_tensor("inp", [128, H, W, D], _F32, kind="ExternalInput").ap()
    out = nc.dram_tensor(
        "out",
        [128, H // 2, W // 2, D],
        _F16 if _OUT_F16 else _F32,
        kind="ExternalOutput",
    ).ap()
    with tile.TileContext(nc) as tc:
        _kernel_body(tc, out, inp, H, W, D)
    nc.compile()
    return nc


_NC_CACHE = {}


def _get_nc():
    if "nc" not in _NC_CACHE:
        _NC_CACHE["nc"] = build_nc()
    return _NC_CACHE["nc"]


def kernel(inp, kernel_size=2, routing_iteration=3, _trace=False, _tmpdir=None):
    inp = np.asarray(inp, dtype=np.float32)
    assert int(kernel_size) == 2, "kernel compiled for kernel_size=2"
    assert inp.shape == (_B, _C, _H, _W, _D), inp.shape
    # routing_iteration is mathematically irrelevant (softmax over singleton
    # axis -> coefficients identically 1); any value >= 1 gives this output.

    nc = _get_nc()
    bpc = _B // _NCORES  # batches per core
    in_maps = [
        {"inp": np.ascontiguousarray(inp[i * bpc : (i + 1) * bpc]).reshape(128, _H, _W, _D)}
        for i in range(_NCORES)
    ]
    res = run_bass_kernel_spmd(
        nc, in_maps, core_ids=list(range(_NCORES)), trace=_trace, tmpdir=_tmpdir
    )
    out = np.empty((_B, _C, _H // 2, _W // 2, _D), dtype=np.float32)
    for i in range(_NCORES):
        out[i * bpc : (i + 1) * bpc] = (
            res.results[i]["out"]
            .astype(np.float32)
            .reshape(bpc, _C, _H // 2, _W // 2, _D)
        )
    if _trace:
        return out, res
    return out


# revision 13
# speedup vs baseline: 1.0732x; 1.0732x over previous
"""Capsule routing pooling kernel for Trainium2 (8 NeuronCores, data parallel).

Math: the reference's softmax is over a singleton axis, so the routing
coefficients are identically 1.0 and the routing iterations never affect the
output.  The computation reduces to, per (b, c, 2x2 spatial tile):
    s   = sum of the four D=16 vectors in the tile
    sq  = sum_d s_d^2
    out = s * sq / ((1 + sq) * (sqrt(sq) + 1e-8)) = s * sqrt(sq) / (1 + sq)

Sharding: batch dim (16) split across 8 cores -> 2 batches/core.  Per core the
(2*64)=128 (b,c) pairs map onto the 128 SBUF partitions; each partition owns a
full 64x64x16 image.

Per-core pipeline (super-groups of row-pairs, schedule [8,8,8,4,2,1,1]):
  - 4 input rows per 2 MB load (contiguous per partition, HWDGE on nc.sync)
  - row-pair add (fp32 in -> fp16 out, DVE 1x) then column-pair add (fp16,
    DVE 2x packed mode) -- fp16 intermediates halve most DVE work vs fp32
    and are ~10x more precise than bf16 for this value range
  - sum over D via a 4-level pairwise tree add (3 fp16 2x levels + 1 fp32
    level) instead of tensor_reduce, which is hard-capped at 1x on DVE
  - square on ACT (fp16 SBUF out; PSUM would cap downstream DVE at 1x),
    emitted immediately after each super-group's col add so ACT always runs
    one super-group ahead of the DVE tail work that consumes it
  - scale chain: sqrt + (1+sq) on ACT, reciprocal + mul on DVE
  - final broadcast multiply on DVE (1x; broadcast operand blocks packing),
    fp16 output stored via the nc.scalar HWDGE ring; host upcasts to fp32
    (exact re-encoding, values unchanged) -- halves store HBM traffic
  - each super-group's tail is emitted one super-group late so the in-order
    DVE stream never waits on ACT's square at the boundary; small tail
    super-groups shorten the post-last-load drain.
"""

import numpy as np

import concourse.bass as bass
import concourse.bacc as bacc
import concourse.tile as tile
from concourse import mybir
from concourse.bass_utils import run_bass_kernel_spmd

_B, _C, _H, _W, _D = 16, 64, 64, 64, 16
_NCORES = 8
_F32 = mybir.dt.float32
_F16 = mybir.dt.float16
_OUT_F16 = True  # store fp16 on device, exact upcast to fp32 on host


def _kernel_body(tc, out_ap, in_ap, H, W, D):
    nc = tc.nc
    P = 128
    nH, nW = H // 2, W // 2
    out_dt = _F16 if _OUT_F16 else _F32

    inv4 = in_ap.rearrange("p (q four) w d -> p q (four w d)", four=4)
    inv2 = in_ap.rearrange("p (rp two) w d -> p rp (two w d)", two=2)
    outv = out_ap.rearrange("p y x d -> p y (x d)")

    # super-group schedule in row-pair units: small batches at both ends
    # (fast pipeline fill / short drain tail), big in the middle
    if nH >= 32:
        sched = [8] * ((nH - 16) // 8) + [8, 4, 2, 1, 1]
    elif nH >= 8:
        sched = [8] * (nH // 8)
    else:
        sched = [nH]
    assert sum(sched) == nH
    nsg_max = max(sched) * nW

    import contextlib

    with contextlib.ExitStack() as ctx:
        slabs = ctx.enter_context(tc.tile_pool(name="slabs", bufs=6))
        rpool = ctx.enter_context(tc.tile_pool(name="rpool", bufs=2))
        mid = ctx.enter_context(tc.tile_pool(name="mid", bufs=3))
        sqp = ctx.enter_context(tc.tile_pool(name="sqp", bufs=2))
        treep = ctx.enter_context(tc.tile_pool(name="treep", bufs=2))
        small = ctx.enter_context(tc.tile_pool(name="small", bufs=2))
        outp = ctx.enter_context(tc.tile_pool(name="outp", bufs=2))

        # one queued tail per super-group, emitted one SG late so the DVE
        # instruction stream never waits on ACT's square at SG boundaries
        pending = []

        def emit_front(sg, g0, fine=False):
            """loads + row-pair adds + column-pair adds for one super-group
            of `sg` row-pairs starting at output row g0.  fine=True loads one
            row-pair per DMA (1 MB) for fast pipeline fill."""
            s_sg = mid.tile([P, sg, nW, D], _F16, tag="s_sg")
            for ci in range(0, sg, 4):
                cg = min(4, sg - ci)  # row-pairs this col-add batch
                r = rpool.tile([P, 4, nW, 2, D], _F16, tag="r")
                for li in range(0, cg, 2):
                    if fine:
                        for q in range(min(2, cg - li)):
                            rp = g0 + ci + li + q
                            slab = slabs.tile([P, 1, 2, nW, 2, D], _F32, tag="slab")
                            nc.sync.dma_start(
                                out=slab[:],
                                in_=inv2[:, rp, :].rearrange(
                                    "p (two b) -> p two b", two=2
                                ),
                            )
                            nc.vector.tensor_add(
                                r[:, li + q : li + q + 1, :, :, :],
                                slab[:, :, 0, :, :, :],
                                slab[:, :, 1, :, :, :],
                            )
                        continue
                    t = (g0 + ci + li) // 2
                    slab = slabs.tile([P, 2, 2, nW, 2, D], _F32, tag="slab")
                    nc.sync.dma_start(
                        out=slab[:],
                        in_=inv4[:, t, :].rearrange(
                            "p (a two b) -> p a two b", a=2, two=2
                        ),
                    )
                    # row-pair sums for 2 row-pairs (DVE 1x: fp32 src)
                    nc.vector.tensor_add(
                        r[:, li : li + 2, :, :, :],
                        slab[:, :, 0, :, :, :],
                        slab[:, :, 1, :, :, :],
                    )
                # column-pair add for cg row-pairs (DVE 2x: all fp16)
                nc.vector.tensor_add(
                    s_sg[:, ci : ci + cg, :, :],
                    r[:, 0:cg, :, 0, :],
                    r[:, 0:cg, :, 1, :],
                )
            return s_sg

        def emit_square(s_sg, sg):
            """ACT square for one super-group, emitted right after its front
            so ACT runs a full super-group ahead of the DVE tail."""
            nsg = sg * nW
            sv = s_sg[:].rearrange("p s x d -> p (s x) d")
            s2 = sqp.tile([P, nsg_max, D], _F16, tag="s2")
            nc.scalar.activation(
                s2[:, 0:nsg, :], sv, mybir.ActivationFunctionType.Square
            )
            return s2

        def tree_reduce(s2, nsg):
            """sum over D=16 via pairwise tree: 3 fp16 2x levels + 1 fp32."""
            t1 = treep.tile([P, nsg_max, 8], _F16, tag="t1")
            t2 = treep.tile([P, nsg_max, 4], _F16, tag="t2")
            t3 = treep.tile([P, nsg_max, 2], _F16, tag="t3")
            nc.vector.tensor_add(
                t1[:, 0:nsg, :], s2[:, 0:nsg, 0:8], s2[:, 0:nsg, 8:16]
            )
            nc.vector.tensor_add(
                t2[:, 0:nsg, :], t1[:, 0:nsg, 0:4], t1[:, 0:nsg, 4:8]
            )
            nc.vector.tensor_add(
                t3[:, 0:nsg, :], t2[:, 0:nsg, 0:2], t2[:, 0:nsg, 2:4]
            )
            return t3

        def chain_slots(nsg):
            ch = small.tile([P, nsg_max, 4], _F32, tag="ch")
            return (
                ch[:, 0:nsg, 0:1],  # sq
                ch[:, 0:nsg, 1:2],  # c1 = 1 + sq
                ch[:, 0:nsg, 2:3],  # a = sqrt(sq)
                ch[:, 0:nsg, 3:4],  # rec = 1/(1+sq); then sc = a*rec
            )

        def emit_tail(sg, g0, s_sg, s2):
            """tree + squash scale + final multiply + store for one SG."""
            nsg = sg * nW
            sv = s_sg[:].rearrange("p s x d -> p (s x) d")
            t3 = tree_reduce(s2, nsg)
            sq, c1, a, rec = chain_slots(nsg)
            nc.vector.tensor_add(sq, t3[:, 0:nsg, 0:1], t3[:, 0:nsg, 1:2])
            nc.scalar.add(c1, sq, 1.0)
            nc.scalar.activation(a, sq, mybir.ActivationFunctionType.Sqrt)
            nc.vector.reciprocal_approx_fast(rec, c1)
            sc = c1  # reuse the c1 slot for the final scale
            nc.vector.tensor_mul(sc, a, rec)
            out_t = outp.tile([P, nsg_max, D], out_dt, tag="out")
            nc.vector.tensor_mul(
                out_t[:, 0:nsg, :], sv, sc.to_broadcast((P, nsg, D))
            )
            nc.scalar.dma_start(
                out=outv[:, g0 : g0 + sg, :],
                in_=out_t[:, 0:nsg, :].rearrange("p n d -> p (n d)"),
            )

        def emit_tail2(tp1, tp2):
            """the last two tails, op-interleaved so ACT and DVE pipeline
            instead of ping-ponging through two serial chains."""
            (sg1, g01, s1, q1), (sg2, g02, s2_, q2) = tp1, tp2
            n1, n2 = sg1 * nW, sg2 * nW
            sv1 = s1[:].rearrange("p s x d -> p (s x) d")
            sv2 = s2_[:].rearrange("p s x d -> p (s x) d")
            u3 = tree_reduce(q1, n1)
            sqa, c1a, aa, reca = chain_slots(n1)
            nc.vector.tensor_add(sqa, u3[:, 0:n1, 0:1], u3[:, 0:n1, 1:2])
            nc.scalar.add(c1a, sqa, 1.0)
            nc.scalar.activation(aa, sqa, mybir.ActivationFunctionType.Sqrt)
            v3 = tree_reduce(q2, n2)
            sqb, c1b, ab, recb = chain_slots(n2)
            nc.vector.tensor_add(sqb, v3[:, 0:n2, 0:1], v3[:, 0:n2, 1:2])
            nc.vector.reciprocal_approx_fast(reca, c1a)
            nc.scalar.add(c1b, sqb, 1.0)
            nc.scalar.activation(ab, sqb, mybir.ActivationFunctionType.Sqrt)
            sca = c1a
            nc.vector.tensor_mul(sca, aa, reca)
            o1 = outp.tile([P, nsg_max, D], out_dt, tag="out")
            nc.vector.tensor_mul(
                o1[:, 0:n1, :], sv1, sca.to_broadcast((P, n1, D))
            )
            nc.scalar.dma_start(
                out=outv[:, g01 : g01 + sg1, :],
                in_=o1[:, 0:n1, :].rearrange("p n d -> p (n d)"),
            )
            nc.vector.reciprocal_approx_fast(recb, c1b)
            scb = c1b
            nc.vector.tensor_mul(scb, ab, recb)
            o2 = outp.tile([P, nsg_max, D], out_dt, tag="out")
            nc.vector.tensor_mul(
                o2[:, 0:n2, :], sv2, scb.to_broadcast((P, n2, D))
            )
            nc.scalar.dma_start(
                out=outv[:, g02 : g02 + sg2, :],
                in_=o2[:, 0:n2, :].rearrange("p n d -> p (n d)"),
            )

        g0 = 0
        last = len(sched) - 1
        for si, sg in enumerate(sched):
            fine = len(sched) > 2 and (si == 0 or si >= len(sched) - 2)
            front = emit_front(sg, g0, fine=fine)
            s2 = emit_square(front, sg)
            if pending and si < last:
                emit_tail(*pending.pop(0))
            pending.append((sg, g0, front, s2))
            g0 += sg
        if len(pending) == 2:
            emit_tail2(pending[0], pending[1])
        else:
            for t in pending:
                emit_tail(*t)


def build_nc(H=_H, W=_W, D=_D):
    """Build and compile the per-core Bass program."""
    nc = bacc.Bacc("TRN2", target_bir_lowering=False, debug=False)
    inp = nc.dram_tensor("inp", [128, H, W, D], _F32, kind="ExternalInput").ap()
    out = nc.dram_tensor(
        "out",
        [128, H // 2, W // 2, D],
        _F16 if _OUT_F16 else _F32,
        kind="ExternalOutput",
    ).ap()
    with tile.TileContext(nc) as tc:
        _kernel_body(tc, out, inp, H, W, D)
    nc.compile()
    return nc


_NC_CACHE = {}


def _get_nc():
    if "nc" not in _NC_CACHE:
        _NC_CACHE["nc"] = build_nc()
    return _NC_CACHE["nc"]


def kernel(inp, kernel_size=2, routing_iteration=3, _trace=False, _tmpdir=None):
    inp = np.asarray(inp, dtype=np.float32)
    assert int(kernel_size) == 2, "kernel compiled for kernel_size=2"
    assert inp.shape == (_B, _C, _H, _W, _D), inp.shape
    # routing_iteration is mathematically irrelevant (softmax over singleton
    # axis -> coefficients identically 1); any value >= 1 gives this output.

    nc = _get_nc()
    bpc = _B // _NCORES  # batches per core
    in_maps = [
        {"inp": np.ascontiguousarray(inp[i * bpc : (i + 1) * bpc]).reshape(128, _H, _W, _D)}
        for i in range(_NCORES)
    ]
    res = run_bass_kernel_spmd(
        nc, in_maps, core_ids=list(range(_NCORES)), trace=_trace, tmpdir=_tmpdir
    )
    out = np.empty((_B, _C, _H // 2, _W // 2, _D), dtype=np.float32)
    for i in range(_NCORES):
        out[i * bpc : (i + 1) * bpc] = (
            res.results[i]["out"]
            .astype(np.float32)
            .reshape(bpc, _C, _H // 2, _W // 2, _D)
        )
    if _trace:
        return out, res
    return out
